# revision 22
# baseline (speedup 1.0000x reference)
"""Trainium2 Bass kernel: 3D bilateral filter (5x5x5, replicate pad).

Reference math (normalization of wd cancels in the final ratio):
    out(v) = sum_k g_k * exp(-a*(p_k - c)^2) * p_k / sum_k g_k * exp(-a*(p_k - c)^2)
with a = 1/(2*0.8^2), g the separable 5x5x5 gaussian, p_k the 125
replicate-padded shifted neighbours and c the center voxel.

Device strategy (per core, data-parallel over the 144 (c,d) planes, 18 each):
  - taps-on-partitions layout: im2col DMA materializes P[125, 2304] fp16 per
    PLANE (one DMA per plane; fp16 halves HBM traffic vs f32)
  - the center row is partition 0 of the P tile (taps permuted center-first)
  - per 768-voxel block (3 per plane):
      PE: D = P - 1*center via a single (I - e0*1^T) fp16 matmul into PSUM
      ACT: E = Derivative_Erf(sqrt(a)*D) = (2/sqrt(pi))*exp(-a*D^2), fp16 --
           the 2/sqrt(pi) factor cancels in num/den
      DVE: T = E*P fp16 (2x_1p perf mode, the only per-block DVE op)
      PE: g-weighted reduction over taps: gwin sliding-window fp16 matmuls
          accumulate [T | E] into row b of a persistent PSUM accumulator
          (gwin carries the gaussian tap weights g_k, so no exp-bias needed)
  - epilogue: out = num * reciprocal(den), one DMA back to DRAM

All replicate padding AND the im2col layout are done host-side:
inh[o, k, :] is the 48x48 valid-region view of output plane o shifted by tap
k = (i, j, l) (replicate-padded), flattened to 2304 contiguous fp16 values.
"""

import math
from contextlib import ExitStack

import numpy as np

import concourse.bass as bass
import concourse.mybir as mybir
import concourse.tile as tile
from concourse import bacc
from concourse.bass_utils import run_bass_kernel_spmd

F32 = mybir.dt.float32
F16 = mybir.dt.float16

SIGMA = 0.8
SQRT_A = 1.0 / (SIGMA * math.sqrt(2.0))  # sqrt(1/(2*sigma^2)) = 0.8838834764
KS = 5
NTAP = KS * KS * KS  # 125
NCORES = 8
C_, D_, H_, W_ = 3, 48, 48, 48
PPC = (C_ * D_) // NCORES  # 18 planes per core
RPB = 16  # output rows per block
NBI = H_ // RPB  # 3 blocks per plane
V = RPB * W_  # 768 free elements per block
NBLK = PPC * NBI  # 54 blocks per core
PLANE_V = H_ * W_  # 2304 valid voxels per plane
# taps reordered host-side so the center tap (2,2,2) sits on partition 0
TAP_PERM = [62] + [k for k in range(NTAP) if k != 62]


def _gauss() -> np.ndarray:
    """The normalized separable gaussian, float64 [125]."""
    sig = [0.3 * ((k - 1) * 0.5 - 1.0) + 0.8 for k in (KS, KS, KS)]
    grids = np.meshgrid(*[np.arange(k) for k in (KS, KS, KS)], indexing="ij")
    ker = np.ones((KS, KS, KS), dtype=np.float64)
    for k, s, m in zip((KS, KS, KS), sig, grids):
        mean = (k - 1) / 2.0
        ker = ker * np.exp(-((m - mean) ** 2) / (2.0 * s * s))
    ker = ker / ker.sum()
    return ker.reshape(-1)


def _epilogue(nc, epi_pool, acc, outp, lo: int, n: int):
    """Drain blocks [lo, lo+n): out = num/den from acc num/den column halves."""
    recip_t = epi_pool.tile([NBLK, V], F32, tag="recip")
    nc.vector.reciprocal(recip_t[lo : lo + n, :], acc[lo : lo + n, V : 2 * V])
    out_t = epi_pool.tile([NBLK, V], F32, tag="out")
    nc.vector.tensor_mul(
        out_t[lo : lo + n, :], acc[lo : lo + n, 0:V], recip_t[lo : lo + n, :]
    )
    dst = outp.rearrange("o (b r) w -> (o b) (r w)", b=NBI)
    nc.sync.dma_start(dst[lo : lo + n, :], out_t[lo : lo + n, :])


def _kernel_body(
    ctx: ExitStack,
    tc: "tile.TileContext",
    inh,
    dmat,
    gwin,
    outp,
    repeat: int = 1,
):
    nc = tc.nc

    consts = ctx.enter_context(tc.tile_pool(name="consts", bufs=1))
    p_pool = ctx.enter_context(tc.tile_pool(name="p", bufs=6))
    rhs_pool = ctx.enter_context(tc.tile_pool(name="rhs", bufs=6))
    epi_pool = ctx.enter_context(tc.tile_pool(name="epi", bufs=1))
    psc_pool = ctx.enter_context(tc.tile_pool(name="psc", bufs=2, space="PSUM"))
    acc_pool = ctx.enter_context(tc.tile_pool(name="acc", bufs=1, space="PSUM"))

    # dmat[p, k] = delta(p,k) - delta(p,0): lhsT computing D = P - center row
    dmat_t = consts.tile([NTAP, NTAP], F16)
    nc.sync.dma_start(dmat_t[:], dmat[:])
    # gwin[p, 64] == g_perm[p] else 0: sliding window gwin[:, 64-b:128-b] is
    # the lhsT that routes block b's g-weighted tap-reduction into PSUM row b
    gwin_t = consts.tile([NTAP, 128], F16)
    nc.sync.dma_start(gwin_t[:], gwin[:])

    # persistent accumulator: row b = [num | den] of block b, contiguous
    # 1536 cols = 3 PSUM banks, one accumulation chain across all of them.
    acc = acc_pool.tile([128, 1536], F32)

    # PE matmuls only support a single sync-wait: consume the const-DMA
    # semaphores with throwaway matmuls so real ones wait on one producer only
    nc.tensor.matmul(
        acc[0:1, 0:1], dmat_t[:, 0:1], dmat_t[:, 0:1],
        start=True, stop=True, skip_group_check=True,
    )
    nc.tensor.matmul(
        acc[0:1, 0:1], gwin_t[:, 0:1], gwin_t[:, 0:1],
        start=True, stop=True, skip_group_check=True,
    )

    for _rep in range(repeat):
      for o in range(PPC):
        # --- im2col load: one DMA per plane, P[k, v] = inh[o, perm[k], v]
        p_t = p_pool.tile([NTAP, PLANE_V], F16)
        nc.sync.dma_start(p_t[:], inh[o])

        for bi in range(NBI):
            b = o * NBI + bi
            n0 = bi * V

            # --- D = P - center (PSUM f32), one fp16 matmul per 512-col chunk
            d_t = psc_pool.tile([NTAP, V], F32)
            for m0 in range(0, V, 512):
                m1 = min(m0 + 512, V)
                nc.tensor.matmul(
                    d_t[:, m0:m1],
                    dmat_t[:],
                    p_t[:, n0 + m0 : n0 + m1],
                    start=True,
                    stop=True,
                )

            # --- one [T | E] tile: DVE writes T into cols 0:768, ACT writes
            # E into cols 768:1536 -- the tap-reduce then runs as three
            # contiguous 512-col matmuls of a single accumulation chain
            te_t = rhs_pool.tile([NTAP, 2 * V], F16, tag="te")
            nc.scalar.activation(
                te_t[:, V : 2 * V],
                d_t[:],
                mybir.ActivationFunctionType.Derivative_Erf,
                scale=SQRT_A,
            )
            # --- T = E * P (fp16 all-SBUF packed: DVE 2x_1p perf mode)
            nc.vector.tensor_mul(
                te_t[:, 0:V], te_t[:, V : 2 * V], p_t[:, n0 : n0 + V]
            )

            # --- reduce taps (g-weighted): [num | den] of block b -> acc row b
            for c0 in range(0, 2 * V, 512):
                nc.tensor.matmul(
                    acc[0:64, c0 : c0 + 512],
                    gwin_t[:, 64 - b : 128 - b],
                    te_t[:, c0 : c0 + 512],
                    start=(b == 0),
                    stop=(b == NBLK - 1),
                )

      # --- epilogue: out = num / den
      _epilogue(nc, epi_pool, acc, outp, 0, NBLK)


def build_program(repeat: int = 1) -> bass.Bass:
    nc = bacc.Bacc("TRN2", target_bir_lowering=False, debug=False)
    inh = nc.declare_dram_parameter("inh", [PPC, NTAP, PLANE_V], F16, isOutput=False)
    dmat = nc.declare_dram_parameter("dmat", [NTAP, NTAP], F16, isOutput=False)
    gwin = nc.declare_dram_parameter("gwin", [NTAP, 128], F16, isOutput=False)
    outp = nc.declare_dram_parameter("out", [PPC, H_, W_], F32, isOutput=True)
    with tile.TileContext(nc) as tc, ExitStack() as ctx:
        _kernel_body(ctx, tc, inh, dmat, gwin, outp, repeat=repeat)
    nc.compile()
    return nc


def build_host_inputs(x: np.ndarray) -> list[dict[str, np.ndarray]]:
    """x: [1, 3, 48, 48, 48] float32 -> per-core in_maps."""
    x = np.asarray(x).reshape(C_, D_, H_, W_).astype(np.float32)
    xp = np.pad(x, ((0, 0), (0, 0), (2, 2), (2, 2)), mode="edge")  # [3,48,52,52]
    xp16 = xp.astype(np.float16)
    dmat = np.eye(NTAP, dtype=np.float16)
    dmat[0, :] -= 1.0
    gwin = np.zeros((NTAP, 128), dtype=np.float16)
    gwin[:, 64] = _gauss()[TAP_PERM]
    in_maps = []
    for m in range(NCORES):
        inh = np.empty((PPC, NTAP, PLANE_V), dtype=np.float16)
        for o in range(PPC):
            q = m * PPC + o
            c, d = divmod(q, D_)
            for i in range(KS):
                dd = min(max(d + i - 2, 0), D_ - 1)
                win = np.lib.stride_tricks.sliding_window_view(
                    xp16[c, dd], (H_, W_)
                )  # [5, 5, 48, 48]
                inh[o, i * 25 : (i + 1) * 25] = win.reshape(25, PLANE_V)
            inh[o] = inh[o, TAP_PERM]
        in_maps.append({"inh": inh, "dmat": dmat, "gwin": gwin})
    return in_maps


_PROGRAM: bass.Bass | None = None


def _get_program() -> bass.Bass:
    global _PROGRAM
    if _PROGRAM is None:
        _PROGRAM = build_program()
    return _PROGRAM


def kernel(x: np.ndarray) -> np.ndarray:
    nc = _get_program()
    in_maps = build_host_inputs(x)
    res = run_bass_kernel_spmd(nc, in_maps, list(range(NCORES)))
    planes = np.concatenate(
        [res.results[m]["out"].reshape(PPC, H_, W_) for m in range(NCORES)], axis=0
    )  # [144, 48, 48]
    return planes.reshape(1, C_, D_, H_, W_).astype(np.float32)
